# revision 21
# baseline (speedup 1.0000x reference)
"""Trainium2 Bass kernel for nn_DigitCapsules (dynamic-routing capsule layer).

Data-parallel over batch B=128 across 8 NeuronCores (BB=16 each); dc_w
replicated.  Per core:

Phase 1 (u generation): u[d,bb,n,o] = sum_i x[bb,n,i] w[d,n,i,o] on the PE via
block-diagonal x stationaries (8 n's per matmul, K=64).  Two matmuls run
concurrently via row tiling (rows 0-63 / 64-127 hold stationaries for even/odd
g).  PSUM banks hold (d, o, g2) so ACT drains land u in SBUF as
u2[p=(nn,bb), f=(d, o, g)] fp16 — the o-MIDDLE layout lets BOTH routing
multiplies run in the DVE 2x perf mode (stride-1 innermost over g for every
operand).  s0 = sum_n u accumulates on the PE (ones-matmuls over drained
chunks), so iteration 0 costs no DVE time.

Routing (3 iters, b2 = u.(v0+v1) so no b state is kept):
  mult1  btmp = u2 * vrepG (v replicated x8 over g)          DVE 2x
  fold-o 16->8->4->2->1 tree over the o middle dim           DVE 2x (last 1x)
  exp    e = exp(b [- M*]) on ACT; iter1 needs no shift (|b1|<2); iter2 gets
         the exact per-(bb,d) group max via PE-transpose of the per-partition
         row maxes, folded into the ACT bias (10 exps, one per d), with
         Z accumulated for free via accum_out.
  mult2  btmp2 = u2 * e (e broadcast over o = stride-0 middle) DVE 2x
  g-fold s[bb,(d,o)] = sum_{g,nn} btmp2 on the PE: 144 accumulating
         ones-matmuls into one PSUM bank (moving = strided (d,o) slice per g).
  squash + v-replication via e8 ones-matmul; Z via ones-matmul on zp.
"""

import numpy as np

import concourse.bacc as bacc
import concourse.bass as bass
import concourse.tile as tile
from concourse import mybir
from concourse.bass_utils import run_bass_kernel_spmd

F16 = mybir.dt.float16
F32 = mybir.dt.float32
AF = mybir.ActivationFunctionType
AX = mybir.AxisListType.X

D, B, N, I, O = 10, 128, 1152, 8, 16
NCORES = 8
BB = B // NCORES      # 16
NN = 8                # n's per matmul group
G = N // NN           # 144 groups
DO = D * O            # 160
OG = O * G            # 2304
FU = D * O * G        # 23040 u elements per partition, layout (d, o, g)
NCH = 9               # DMA chunks
GCH = G // NCH        # 16 g per chunk
CH1 = 48              # mult1 chunk g's (multiple of 8 for vrepG)
CH2 = 36              # mult2 chunk g's


def _ap(t, dims, offset=0, p=None):
    base = t[:] if p is None else t[p[0]:p[1], :]
    return bass.AP(tensor=base.tensor, offset=base.offset + offset,
                   ap=[base.ap[0]] + [list(d) for d in dims])


def build_nc(debug=False, it2_shift=True, rowsplit=False, stridedpsum=True):
    nc = bacc.Bacc(None, target_bir_lowering=False)

    if rowsplit:
        xb2_d = nc.dram_tensor("xb2", [128, (G // 2) * 128], F16, kind="ExternalInput")
        wp2_d = nc.dram_tensor("wp2", [128, (G // 2) * DO], F16, kind="ExternalInput")
    else:
        xb2_d = nc.dram_tensor("xb2", [64, G * 128], F16, kind="ExternalInput")
        wp2_d = nc.dram_tensor("wp2", [64, G * DO], F16, kind="ExternalInput")
    eones_d = nc.dram_tensor("eones", [128, 16], F32, kind="ExternalInput")
    e8_d = nc.dram_tensor("e8", [16, 128], F32, kind="ExternalInput")
    eye_d = nc.dram_tensor("eye", [128, 128], F32, kind="ExternalInput")
    out_d = nc.dram_tensor("out", [D, BB, O], F32, kind="ExternalOutput")
    if debug:
        dbg_u2 = nc.dram_tensor("dbg_u2", [128, FU], F16, kind="ExternalOutput")
        dbg_b = nc.dram_tensor("dbg_b", [2, 128, D * G], F32, kind="ExternalOutput")
        dbg_ev = nc.dram_tensor("dbg_ev", [2, 128, D * G], F16, kind="ExternalOutput")
        dbg_sm = nc.dram_tensor("dbg_sm", [3, 16, DO], F32, kind="ExternalOutput")

    with tile.TileContext(nc) as tc:
        with (
            tc.tile_pool(name="const", bufs=1) as const,
            tc.tile_pool(name="big", bufs=1) as big,
            tc.tile_pool(name="stream", bufs=3) as stream,
            tc.tile_pool(name="bt1p", bufs=2) as bt1p,
            tc.tile_pool(name="bt2p", bufs=2) as bt2p,
            tc.tile_pool(name="psacc", bufs=1, space="PSUM") as psacc,
            tc.tile_pool(name="psmisc", bufs=2, space="PSUM") as psmisc,
        ):
            eones = const.tile([128, 16], F32)
            nc.sync.dma_start(eones[:], eones_d[:])
            e8t = const.tile([16, 128], F32)
            nc.sync.dma_start(e8t[:], e8_d[:])
            eye = const.tile([128, 128], F32)
            nc.sync.dma_start(eye[:], eye_d[:])
            eones16 = const.tile([128, 16], F16)
            nc.scalar.copy(eones16[:], eones[:])

            u2 = big.tile([128, FU], F16)       # (d, o, g)
            fb1 = big.tile([128, D * 8 * G], F16)   # (d, o8, g)
            fb2 = big.tile([128, D * 4 * G], F16)
            fb3 = big.tile([128, D * 2 * G], F16)
            b32 = big.tile([128, D * G], F32)   # (d, g)
            ev = big.tile([128, D * G], F16)    # (d, g)
            zp = big.tile([128, 16], F32)
            vrepG = big.tile([128, DO * 8], F16)  # (d, o, g8)
            mrow = big.tile([128, 16], F32)
            mrowT = big.tile([10, 128], F32)
            mT = big.tile([10, 16], F32)
            negM16 = big.tile([16, 16], F32)
            negM = big.tile([128, 16], F32)
            warm = big.tile([128, 16], F16)
            sm = big.tile([16, DO], F32)
            smf = big.tile([16, DO], F32)
            sq = big.tile([16, DO], F32)
            p1 = big.tile([16, DO], F32)
            rden = big.tile([16, DO], F32)
            rr = big.tile([16, DO], F32)
            tt = big.tile([16, DO], F32)
            vv0 = big.tile([16, DO], F32)
            vv1 = big.tile([16, DO], F32)
            vv2 = big.tile([16, DO], F32)
            vacc = big.tile([16, DO], F32)
            zf = big.tile([16, 16], F32)
            rz = big.tile([16, 16], F32)

            nc.vector.memset(zp[:], 0.0)
            # Pre-trigger the exp table load so routing doesn't pay it.
            nc.scalar.activation(warm[:], zp[:], AF.Exp)

            # ---------------- phase 1: u2 generation + s0 ----------------
            # s0 accumulates as 4 g-lanes (psum columns follow the moving
            # free order (d, o, g2) per accumulator); combined at iter 0.
            psA = psacc.tile([16, 2 * DO], F32, tag="accA")
            psB = psacc.tile([16, 2 * DO], F32, tag="accB")
            ngrp = G // 4           # drain-groups of 4 g's (2 banks)

            def emit_s0(idx, last):
                nc.tensor.matmul(
                    psA[:], eones16[:],
                    _ap(u2, [[OG, D], [G, O], [1, 2]], offset=idx * 4),
                    start=(idx == 0), stop=last,
                )
                nc.tensor.matmul(
                    psB[:], eones16[:],
                    _ap(u2, [[OG, D], [G, O], [1, 2]], offset=idx * 4 + 2),
                    start=(idx == 0), stop=last,
                )

            with tc.tile_pool(name="pmm", bufs=2, space="PSUM") as pmm:
                for ch in range(NCH):
                    nb = GCH // 2   # column blocks (g-pairs) per chunk
                    if rowsplit:
                        xch = stream.tile([128, nb * 128], F16, tag="xch")
                        wch = stream.tile([128, nb * DO], F16, tag="wch")
                        nc.sync.dma_start(xch[:], xb2_d[:, ch * nb * 128:(ch + 1) * nb * 128])
                        nc.sync.dma_start(wch[:], wp2_d[:, ch * nb * DO:(ch + 1) * nb * DO])
                    else:
                        xch = stream.tile([64, GCH * 128], F16, tag="xch")
                        wch = stream.tile([64, GCH * DO], F16, tag="wch")
                        nc.sync.dma_start(xch[:], xb2_d[:, ch * GCH * 128:(ch + 1) * GCH * 128])
                        nc.sync.dma_start(wch[:], wp2_d[:, ch * GCH * DO:(ch + 1) * GCH * DO])
                    for grp in range(4):        # 2 g-pairs = 4 g's each
                        pg = pmm.tile([128, 1024], F32, tag="pg")
                        for k2 in range(2):
                            if stridedpsum:
                                pdst = [_ap(pg, [[2 * O, D], [2, O]], offset=k2 * 512),
                                        _ap(pg, [[2 * O, D], [2, O]], offset=k2 * 512 + 1)]
                            else:
                                pdst = [_ap(pg, [[O, D], [1, O]], offset=k2 * 512),
                                        _ap(pg, [[O, D], [1, O]], offset=k2 * 512 + DO)]
                            if rowsplit:
                                kl = grp * 2 + k2   # g-pair block index in chunk
                                nc.tensor.matmul(
                                    pdst[0],
                                    _ap(xch, [[1, 128]], offset=kl * 128, p=(0, 64)),
                                    _ap(wch, [[1, DO]], offset=kl * DO, p=(0, 64)),
                                    tile_position=(0, 0),
                                )
                                nc.tensor.matmul(
                                    pdst[1],
                                    _ap(xch, [[1, 128]], offset=kl * 128, p=(64, 128)),
                                    _ap(wch, [[1, DO]], offset=kl * DO, p=(64, 128)),
                                    tile_position=(64, 0),
                                )
                            else:
                                for j in range(2):
                                    gl = (grp * 2 + k2) * 2 + j  # g within chunk
                                    nc.tensor.matmul(
                                        pdst[j],
                                        _ap(xch, [[1, 128]], offset=gl * 128),
                                        _ap(wch, [[1, DO]], offset=gl * DO),
                                    )
                        gi = ch * 4 + grp
                        g0 = gi * 4
                        for k2 in range(2):
                            if stridedpsum:
                                nc.scalar.copy(
                                    _ap(u2, [[OG, D], [G, O], [1, 2]], offset=g0 + k2 * 2),
                                    _ap(pg, [[2 * O, D], [2, O], [1, 2]], offset=k2 * 512),
                                )
                            else:
                                nc.scalar.copy(
                                    _ap(u2, [[1, 2], [OG, D], [144, O]], offset=g0 + k2 * 2),
                                    _ap(pg, [[DO, 2], [O, D], [1, O]], offset=k2 * 512),
                                )
                        # lag-1 s0 accumulation so PE isn't stalled on drains
                        if gi > 0:
                            emit_s0(gi - 1, last=False)
                emit_s0(ngrp - 1, last=True)

            if debug:
                nc.sync.dma_start(dbg_u2[:], u2[:])

            def squash(s_tile, v_tile):
                # v = s*|s|/(1+s^2)
                nc.scalar.activation(sq[:], s_tile[:], AF.Square)
                nc.scalar.activation(p1[:], sq[:], AF.Copy, bias=1.0)
                nc.vector.reciprocal(rden[:], p1[:])
                nc.scalar.activation(rr[:], s_tile[:], AF.Abs)
                nc.vector.tensor_mul(tt[:], s_tile[:], rr[:])
                nc.vector.tensor_mul(v_tile[:], tt[:], rden[:])

            def make_vrepG(v_tile):
                pv = psmisc.tile([128, 512], F32, tag="ms")
                nc.tensor.matmul(pv[:, 0:DO], e8t[:], v_tile[:])
                nc.scalar.copy(
                    _ap(vrepG, [[8 * O, D], [8, O], [1, 8]]),
                    _ap(pv, [[O, D], [1, O], [0, 8]]),
                )

            # ---------------- iteration 0 ----------------
            tA = big.tile([16, 2 * DO], F32)
            tB = big.tile([16, 2 * DO], F32)
            sA = big.tile([16, DO], F32)
            sB = big.tile([16, DO], F32)
            s0f = big.tile([16, DO], F32)
            nc.scalar.copy(tA[:], psA[:])
            nc.scalar.copy(tB[:], psB[:])
            nc.vector.tensor_add(
                sA[:], _ap(tA, [[2, DO]]), _ap(tA, [[2, DO]], offset=1))
            nc.vector.tensor_add(
                sB[:], _ap(tB, [[2, DO]]), _ap(tB, [[2, DO]], offset=1))
            nc.vector.tensor_add(s0f[:], sA[:], sB[:])
            nc.scalar.activation(sm[:], s0f[:], AF.Copy, scale=1.0 / float(N))
            if debug:
                nc.sync.dma_start(dbg_sm[0], sm[:])
            squash(sm, vv0)
            make_vrepG(vv0)

            # ---------------- iterations 1, 2 ----------------
            for it in (1, 2):
                # mult1 + fold-o level 1, chunked by g
                for c in range(G // CH1):
                    bt = bt1p.tile([128, DO * CH1], F16, tag="bt1")
                    nc.vector.tensor_mul(
                        _ap(bt, [[CH1, DO], [8, CH1 // 8], [1, 8]]),
                        _ap(u2, [[G, DO], [8, CH1 // 8], [1, 8]], offset=c * CH1),
                        _ap(vrepG, [[8, DO], [0, CH1 // 8], [1, 8]]),
                    )
                    nc.vector.tensor_add(
                        _ap(fb1, [[8 * G, D], [G, 8], [1, CH1]], offset=c * CH1),
                        _ap(bt, [[16 * CH1, D], [CH1, 8], [1, CH1]]),
                        _ap(bt, [[16 * CH1, D], [CH1, 8], [1, CH1]], offset=8 * CH1),
                    )
                nc.vector.tensor_add(
                    _ap(fb2, [[4 * G, D], [G, 4], [1, G]]),
                    _ap(fb1, [[8 * G, D], [G, 4], [1, G]]),
                    _ap(fb1, [[8 * G, D], [G, 4], [1, G]], offset=4 * G),
                )
                nc.vector.tensor_add(
                    _ap(fb3, [[2 * G, D], [G, 2], [1, G]]),
                    _ap(fb2, [[4 * G, D], [G, 2], [1, G]]),
                    _ap(fb2, [[4 * G, D], [G, 2], [1, G]], offset=2 * G),
                )
                nc.vector.tensor_add(
                    _ap(b32, [[G, D], [1, G]]),
                    _ap(fb3, [[2 * G, D], [1, G]]),
                    _ap(fb3, [[2 * G, D], [1, G]], offset=G),
                )
                if debug:
                    nc.sync.dma_start(dbg_b[it - 1], b32[:])

                if it == 1 or not it2_shift:
                    # |b| is small (<= ~2 after iter 1 on this data); softmax
                    # shift is mathematically optional.
                    nc.scalar.activation(ev[:], b32[:], AF.Exp)
                else:
                    # exact per-(bb,d) group max, folded into the exp bias
                    nc.vector.reduce_max(mrow[:, 0:D], _ap(b32, [[G, D], [1, G]]), axis=AX)
                    ptr = psmisc.tile([128, 512], F32, tag="ms")
                    nc.tensor.transpose(ptr[0:10, 0:128], mrow[:, 0:D], eye[:])
                    nc.scalar.copy(mrowT[:], ptr[0:10, 0:128])
                    nc.vector.reduce_max(mT[:, 0:16], _ap(mrowT, [[1, 16], [16, 8]]), axis=AX)
                    ptr2 = psmisc.tile([128, 512], F32, tag="ms")
                    nc.tensor.transpose(ptr2[0:16, 0:10], mT[:, 0:16], eye[0:10, 0:10])
                    nc.scalar.activation(negM16[:, 0:10], ptr2[0:16, 0:10], AF.Copy, scale=-1.0)
                    pe8 = psmisc.tile([128, 512], F32, tag="ms")
                    nc.tensor.matmul(pe8[:, 0:10], e8t[:], negM16[:, 0:10])
                    nc.scalar.copy(negM[:, 0:10], pe8[:, 0:10])
                    nc.vector.memset(zp[:, 0:D], 0.0)
                    for d in range(D):
                        nc.scalar.activation(
                            _ap(ev, [[1, G]], offset=d * G),
                            _ap(b32, [[1, G]], offset=d * G),
                            AF.Exp, bias=negM[:, d:d + 1],
                            accum_out=zp[:, d:d + 1],
                        )
                if debug:
                    nc.sync.dma_start(dbg_ev[it - 1], ev[:])

                # mult2 + PE g-fold, chunked by g (reuses the s0 psum buffer)
                pf4 = psacc.tile([16, 2 * DO], F32, tag="accA")
                pf = pf4[:, 0:DO]
                for c in range(G // CH2):
                    b2t = bt2p.tile([128, DO * CH2], F16, tag="bt2")
                    nc.vector.tensor_mul(
                        _ap(b2t, [[O * CH2, D], [CH2, O], [1, CH2]]),
                        _ap(u2, [[OG, D], [G, O], [1, CH2]], offset=c * CH2),
                        _ap(ev, [[G, D], [0, O], [1, CH2]], offset=c * CH2),
                    )
                    for g in range(CH2):
                        nc.tensor.matmul(
                            pf[:], eones16[:],
                            _ap(b2t, [[O * CH2, D], [CH2, O]], offset=g),
                            start=(c == 0 and g == 0),
                            stop=(c == G // CH2 - 1 and g == CH2 - 1),
                        )
                if it == 1 or not it2_shift:
                    with nc.allow_low_precision(reason="fp32 accum out"):
                        nc.vector.reduce_sum(zp[:, 0:D], _ap(ev, [[G, D], [1, G]]), axis=AX)

                pzf = psmisc.tile([128, 512], F32, tag="ms")
                nc.tensor.matmul(pzf[0:16, 0:16], eones[:], zp[:])
                nc.scalar.copy(zf[:], pzf[0:16, 0:16])
                nc.vector.reciprocal(rz[:, 0:D], zf[:, 0:D])
                nc.scalar.copy(smf[:], pf[:])
                nc.vector.tensor_mul(
                    _ap(sm, [[O, D], [1, O]]),
                    _ap(smf, [[O, D], [1, O]]),
                    _ap(rz, [[1, D], [0, O]]),
                )
                if debug:
                    nc.sync.dma_start(dbg_sm[it], sm[:])
                vv = vv1 if it == 1 else vv2
                squash(sm, vv)
                if it == 1:
                    nc.vector.tensor_add(vacc[:], vv0[:], vv1[:])
                    make_vrepG(vacc)

            out_ap = bass.AP(tensor=out_d.tensor if hasattr(out_d, "tensor") else out_d,
                             offset=0, ap=[[O, BB], [BB * O, D], [1, O]])
            nc.sync.dma_start(out_ap, vv2[:])

    nc.compile()
    return nc


_NC_CACHE = {}


def _get_nc(**kw):
    key = tuple(sorted(kw.items()))
    if key not in _NC_CACHE:
        _NC_CACHE[key] = build_nc(**kw)
    return _NC_CACHE[key]


def host_prep(x, dc_w, rowsplit=False):
    x = np.asarray(x, np.float32)
    dc_w = np.asarray(dc_w, np.float32)
    wr = dc_w.reshape(D, G, NN, I, O).transpose(2, 3, 1, 0, 4)   # [nn,i,g,d,o]
    wcols = np.ascontiguousarray(wr.reshape(64, G, DO)).astype(np.float16)
    if rowsplit:
        wp2 = np.zeros((128, (G // 2) * DO), np.float16)
        wp2[0:64] = wcols[:, 0::2].reshape(64, -1)
        wp2[64:128] = wcols[:, 1::2].reshape(64, -1)
    else:
        wp2 = np.ascontiguousarray(wcols.reshape(64, -1))
    xb2s = []
    for c in range(NCORES):
        xr = x[c * BB:(c + 1) * BB].reshape(BB, G, NN, I)
        blk = np.zeros((NN, I, G, NN, BB), np.float32)
        for nn in range(NN):
            blk[nn, :, :, nn, :] = xr[:, :, nn, :].transpose(2, 1, 0)
        xcols = blk.reshape(64, G, 128).astype(np.float16)
        if rowsplit:
            xb2 = np.zeros((128, (G // 2) * 128), np.float16)
            xb2[0:64] = xcols[:, 0::2].reshape(64, -1)
            xb2[64:128] = xcols[:, 1::2].reshape(64, -1)
        else:
            xb2 = xcols.reshape(64, -1)
        xb2s.append(np.ascontiguousarray(xb2))
    eones = np.zeros((128, 16), np.float32)
    for nn in range(NN):
        for bb in range(BB):
            eones[nn * BB + bb, bb] = 1.0
    e8 = np.ascontiguousarray(eones.T)
    eye = np.eye(128, dtype=np.float32)
    return wp2, xb2s, eones, e8, eye


def run(x, dc_w, debug=False, it2_shift=True, rowsplit=False, stridedpsum=True,
        **spmd_kwargs):
    wp2, xb2s, eones, e8, eye = host_prep(x, dc_w, rowsplit=rowsplit)
    nc = _get_nc(debug=debug, it2_shift=it2_shift, rowsplit=rowsplit,
                 stridedpsum=stridedpsum)
    in_maps = [
        {"xb2": xb2s[c], "wp2": wp2, "eones": eones, "e8": e8, "eye": eye}
        for c in range(NCORES)
    ]
    res = run_bass_kernel_spmd(nc, in_maps, core_ids=list(range(NCORES)), **spmd_kwargs)
    out = np.zeros((D, B, 1, 1, O), np.float32)
    for c in range(NCORES):
        out[:, c * BB:(c + 1) * BB, 0, 0, :] = res.results[c]["out"]
    return out, res


def kernel(x, dc_w):
    return run(x, dc_w)[0]
